# revision 13
# baseline (speedup 1.0000x reference)
"""APPNP (MLP + 2-step GCN propagation) on 8 Trainium2 NeuronCores.

Strategy (data-parallel over nodes):
 - nodes sharded 12500/core (padded to 12544), natural order.
 - MLP runs transposed (features on partitions) in bf16, PSUM fp32.
 - propagation: g = dinv * h is AllGathered (bf16, rows padded to 256 B)
   into a replicated DRAM table [8*12544, 128]; each core gathers g[src]
   for its edges with batched SWDGE dma_gather (int16 indices).
 - int16 index range forces 4 source-quarter passes (25088 rows each).
   Each pass uses its OWN destination permutation (dsts sorted by that
   quarter's in-count) so the padded-CSR slot schedule stays tight; the
   pass result is merged into a DRAM accumulator with dma_scatter_add
   (indices are core-local, int16-safe), which also un-permutes.
 - h_{k+1} = 0.9*dinv*(acc + g_own) + 0.1*h0 per 128-node tile on DVE.
"""

import numpy as np
import ml_dtypes

N, E, F, H, C = 100000, 1600000, 512, 256, 64
KSTEPS, ALPHA = 2, 0.1
M = 8                       # cores
NQ = 4                      # source quarters (int16 gather range)
GCH = 8                     # gather chunk: 8 cols * 128 = 1024 idxs (HW cap)
BF16 = ml_dtypes.bfloat16


def _derived():
    NSH = N // M                          # real nodes per core
    NSP = ((NSH + 127) // 128) * 128      # padded
    NT = NSP // 128                       # tiles per core
    TBL = M * NSP                         # table rows
    QROWS = TBL // NQ                     # rows per source quarter
    return NSH, NSP, NT, TBL, QROWS


def _wrap16(idx):
    """idx[i] -> [128, ceil(n/16)] int16, entry i at (i%16, i//16),
    replicated across the 8 16-partition stripes."""
    n = idx.size
    cols = (n + 15) // 16
    a = np.zeros((16, cols), dtype=np.int16)
    a[np.arange(n) % 16, np.arange(n) // 16] = idx.astype(np.int16)
    return np.tile(a, (8, 1))


def _host_prep(x, W1, b1, p, W2, b2, edge_index):
    NSH, NSP, NT, TBL, QROWS = _derived()
    src = edge_index[0].astype(np.int64)
    dst = edge_index[1].astype(np.int64)

    deg = (np.bincount(dst, minlength=N) + 1).astype(np.float32)

    # table row of a global src node (natural per-core order, padded)
    src_row = (src // NSH) * NSP + (src % NSH)
    src_q = src_row // QROWS
    src_local = (src_row - src_q * QROWS).astype(np.int64)
    dst_core = dst // NSH
    dst_local = (dst % NSH).astype(np.int64)

    # per-(core, quarter) in-counts  [M, NQ, NSP]
    cnt = np.zeros((M, NQ, NSP), dtype=np.int64)
    np.add.at(cnt, (dst_core, src_q, dst_local), 1)

    # per-(core, quarter) dst permutation: sort by count desc (stable)
    perm = np.empty((M, NQ, NSP), dtype=np.int64)
    csort = np.empty((M, NQ, NSP), dtype=np.int64)
    for c in range(M):
        for q in range(NQ):
            pq = np.argsort(-cnt[c, q], kind="stable")
            perm[c, q] = pq
            csort[c, q] = cnt[c, q][pq]

    # shared slot schedule per pass: S[q][t] = max over cores of the
    # (t*128)-th largest count (sorted desc => max within tile at head)
    S = np.maximum(csort[:, :, ::128].max(axis=0), 1)  # [NQ, NT]
    colstart = np.zeros((NQ, NT + 1), dtype=np.int64)
    colstart[:, 1:] = np.cumsum(S, axis=1)
    TCq = colstart[:, -1]                               # [NQ]

    zero_local = NSP + NSH        # a guaranteed-zero table row per quarter

    # bucket edges by (core, quarter, dst) and emit slot-padded gather idxs
    in_maps = []
    for c in range(M):
        sel = dst_core == c
        sq, sl, dl = src_q[sel], src_local[sel], dst_local[sel]
        gidx_parts = []
        sidx_parts = []
        for q in range(NQ):
            qs = sq == q
            dq, lq = dl[qs], sl[qs]
            pos = np.empty(NSP, dtype=np.int64)
            pos[perm[c, q]] = np.arange(NSP)           # final node -> pass pos
            dpos = pos[dq]                              # pass position of dst
            order = np.argsort(dpos, kind="stable")
            dpos_s, lq_s = dpos[order], lq[order]
            cct = np.bincount(dpos_s, minlength=NSP)
            start = np.concatenate([[0], np.cumsum(cct)])[:-1]
            slot = np.arange(dpos_s.size) - start[dpos_s]
            tiles = dpos_s // 128
            rows = dpos_s % 128
            cols = colstart[q][tiles] + slot
            arr = np.full((int(TCq[q]), 128), zero_local, dtype=np.int64)
            arr[cols, rows] = lq_s                     # idx i = col*128+row
            gidx_parts.append(arr.reshape(-1))
            sidx_parts.append(perm[c, q])              # payload i -> final row
        gidx = _wrap16(np.concatenate(gidx_parts))
        sidx = _wrap16(np.concatenate(sidx_parts))

        lo = c * NSH
        xt = np.zeros((F, NSP), dtype=BF16)
        xt[:, :NSH] = np.ascontiguousarray(x[lo:lo + NSH, :].T).astype(BF16)
        deg_pad = np.ones(NSP, dtype=np.float32)
        deg_pad[:NSH] = deg[lo:lo + NSH]

        in_maps.append({
            "xt": xt,
            "gidx": gidx,
            "sidx": sidx,
            "deg": np.ascontiguousarray(deg_pad.reshape(NT, 128).T),
            "w1": W1.astype(BF16),
            "w2": W2.astype(BF16),
            "pvec": p.astype(np.float32).reshape(2, H // 2).T.copy(),
            "b1": b1.astype(np.float32).reshape(2, H // 2).T.copy(),
            "b2": b2.astype(np.float32).reshape(C, 1).copy(),
            "ident": np.eye(128, dtype=BF16),
            "identf": np.eye(64, dtype=np.float32),
        })

    sched = {
        "S": [[int(v) for v in S[q]] for q in range(NQ)],
        "TCq": [int(v) for v in TCq],
    }
    return in_maps, sched


def _build(sched):
    import concourse.bacc as bacc
    import concourse.mybir as mybir
    import concourse.tile as tile
    from concourse import library_config

    NSH, NSP, NT, TBL, QROWS = _derived()
    S = sched["S"]
    TCq = sched["TCq"]
    GTOT = 128 * sum(TCq)
    fp32 = mybir.dt.float32
    bf16 = mybir.dt.bfloat16
    i16 = mybir.dt.int16

    nc = bacc.Bacc(None, target_bir_lowering=False, num_swdge_queues=4)

    xt_e = nc.declare_dram_parameter("xt", [F, NSP], bf16, isOutput=False)
    gidx_e = nc.declare_dram_parameter("gidx", [128, GTOT // 16], i16, isOutput=False)
    sidx_e = nc.declare_dram_parameter("sidx", [128, NQ * NSP // 16], i16, isOutput=False)
    deg_e = nc.declare_dram_parameter("deg", [128, NT], fp32, isOutput=False)
    w1_e = nc.declare_dram_parameter("w1", [F, H], bf16, isOutput=False)
    w2_e = nc.declare_dram_parameter("w2", [H, C], bf16, isOutput=False)
    p_e = nc.declare_dram_parameter("pvec", [H // 2, 2], fp32, isOutput=False)
    b1_e = nc.declare_dram_parameter("b1", [H // 2, 2], fp32, isOutput=False)
    b2_e = nc.declare_dram_parameter("b2", [C, 1], fp32, isOutput=False)
    id_e = nc.declare_dram_parameter("ident", [128, 128], bf16, isOutput=False)
    idf_e = nc.declare_dram_parameter("identf", [64, 64], fp32, isOutput=False)
    out_e = nc.declare_dram_parameter("out", [NSP, C], fp32, isOutput=True)

    # MLP row chunks
    rcs = []
    off = 0
    while off < NSP:
        w = min(512, NSP - off)
        rcs.append((off, w))
        off += w

    with tile.TileContext(nc) as tc:
        with (
            tc.tile_pool(name="const", bufs=1) as constp,
            tc.tile_pool(name="big", bufs=1) as bigp,
            tc.tile_pool(name="xts", bufs=3) as xtp,
            tc.tile_pool(name="acts", bufs=3) as actp,
            tc.tile_pool(name="gat", bufs=8) as gatp,
            tc.tile_pool(name="epi", bufs=8) as epip,
            tc.tile_pool(name="ps1", bufs=2, space="PSUM") as ps1,
            tc.tile_pool(name="ps2", bufs=2, space="PSUM") as ps2,
            tc.tile_pool(name="pst", bufs=2, space="PSUM") as pst,
            tc.tile_pool(name="psa", bufs=2, space="PSUM") as psa,
            tc.tile_pool(name="dram", bufs=1, space="DRAM") as dramp,
        ):
            nc.gpsimd.load_library(library_config.mlp)

            # ---- constants ----
            w1_sb = constp.tile([128, F // 128, H], bf16)
            nc.sync.dma_start(out=w1_sb[:], in_=w1_e.ap().rearrange("(c p) h -> p c h", p=128))
            w2_sb = constp.tile([128, H // 128, C], bf16)
            nc.sync.dma_start(out=w2_sb[:], in_=w2_e.ap().rearrange("(c p) h -> p c h", p=128))
            id_sb = constp.tile([128, 128], bf16)
            nc.sync.dma_start(out=id_sb[:], in_=id_e[:, :])
            idf_sb = constp.tile([64, 64], fp32)
            nc.sync.dma_start(out=idf_sb[:], in_=idf_e[:, :])
            b2_sb = constp.tile([64, 1], fp32)
            nc.sync.dma_start(out=b2_sb[:], in_=b2_e[:, :])

            gidx_sb = constp.tile([128, GTOT // 16], i16)
            nc.sync.dma_start(out=gidx_sb[:], in_=gidx_e[:, :])
            sidx_sb = constp.tile([128, NQ * NSP // 16], i16)
            nc.sync.dma_start(out=sidx_sb[:], in_=sidx_e[:, :])

            p_sb = constp.tile([H // 2, 2], fp32)
            nc.sync.dma_start(out=p_sb[:], in_=p_e[:, :])
            b1_sb = constp.tile([H // 2, 2], fp32)
            nc.sync.dma_start(out=b1_sb[:], in_=b1_e[:, :])
            pc_sb = constp.tile([H // 2, 2], fp32)
            nc.vector.tensor_scalar(
                out=pc_sb[:], in0=p_sb[:], scalar1=0.0, scalar2=1.0,
                op0=mybir.AluOpType.max, op1=mybir.AluOpType.min)
            pb1_sb = constp.tile([H // 2, 2], fp32)
            nc.vector.tensor_mul(out=pb1_sb[:], in0=pc_sb[:], in1=b1_sb[:])

            deg_sb = constp.tile([128, NT], fp32)
            nc.sync.dma_start(out=deg_sb[:], in_=deg_e[:, :])
            sq_sb = constp.tile([128, NT], fp32)
            nc.scalar.sqrt(out=sq_sb[:], in_=deg_sb[:])
            dinv_sb = constp.tile([128, NT], fp32)
            nc.vector.reciprocal(out=dinv_sb[:], in_=sq_sb[:])
            dinv09_sb = constp.tile([128, NT], fp32)
            nc.vector.tensor_scalar_mul(dinv09_sb[:], dinv_sb[:], 1.0 - ALPHA)

            # ---- persistent big buffers ----
            h0a_sb = bigp.tile([128, NT, C], fp32)      # 0.1 * h0
            g_sb = bigp.tile([128, NT, C], bf16)        # current g (own shard)
            stash = [bigp.tile([128, NT, C], fp32, name=f"stash{i}") for i in range(2)]

            # ---- DRAM bounce + tables + accumulators ----
            bounce = [dramp.tile([NSP, 2 * C], bf16, tag=f"bounce{k}", name=f"bounce{k}") for k in range(2)]
            table = [
                dramp.tile([TBL, 2 * C], bf16, tag=f"table{k}", name=f"table{k}", addr_space="Shared")
                for k in range(2)
            ]
            acc = [dramp.tile([NSP, C], fp32, tag=f"acc{k}", name=f"acc{k}") for k in range(2)]

            zero_sb = constp.tile([128, C], bf16, name="zero_sb")
            nc.vector.memset(zero_sb[:], 0.0)
            zerof_sb = constp.tile([128, C], fp32, name="zerof_sb")
            nc.vector.memset(zerof_sb[:], 0.0)

            # one-time zeroing: bounce pad halves (cols C:2C), pad rows
            for k in range(2):
                for t in range(NT):
                    nc.sync.dma_start(
                        out=bounce[k][:, :].rearrange("(t p) c -> p t c", p=128)[:, t, C:],
                        in_=zero_sb[:])
                nc.sync.dma_start(out=bounce[k][NSH:NSP, :C], in_=zero_sb[:NSP - NSH, :])

            def bounce_tile(k, t):
                rows = min(128, NSH - t * 128)
                if rows <= 0:
                    return
                nc.sync.dma_start(
                    out=bounce[k][t * 128:t * 128 + rows, :C],
                    in_=g_sb[:rows, t, :])

            # ================= MLP =================
            for (off, w) in rcs:
                xt_sb = xtp.tile([128, F // 128, 512], bf16, tag="xt")
                nc.sync.dma_start(
                    out=xt_sb[:, :, :w],
                    in_=xt_e[:, off:off + w].rearrange("(c p) n -> p c n", p=128))
                a_sb = actp.tile([128, H // 128, 512], bf16, tag="a")
                for ht in range(H // 128):
                    pt1 = ps1.tile([128, 512], fp32, tag="pt1")
                    for fc in range(F // 128):
                        nc.tensor.matmul(
                            pt1[:, :w],
                            lhsT=w1_sb[:, fc, ht * 128:(ht + 1) * 128],
                            rhs=xt_sb[:, fc, :w],
                            start=(fc == 0), stop=(fc == F // 128 - 1))
                    nc.scalar.activation(
                        out=a_sb[:, ht, :w], in_=pt1[:, :w],
                        func=mybir.ActivationFunctionType.Relu,
                        bias=pb1_sb[:, ht:ht + 1], scale=pc_sb[:, ht:ht + 1])
                pt2 = ps2.tile([C, 512], fp32, tag="pt2")
                for ht in range(H // 128):
                    nc.tensor.matmul(
                        pt2[:, :w], lhsT=w2_sb[:, ht, :], rhs=a_sb[:, ht, :w],
                        start=(ht == 0), stop=(ht == H // 128 - 1))
                h0t_sb = actp.tile([C, 512], fp32, tag="h0t")
                nc.vector.tensor_scalar_add(h0t_sb[:, :w], pt2[:, :w], b2_sb[:, :1])
                for j in range(w // 128):
                    t = off // 128 + j
                    ptt = pst.tile([128, C], fp32, tag="ptt")
                    nc.tensor.transpose(
                        out=ptt[:], in_=h0t_sb[:, j * 128:(j + 1) * 128],
                        identity=idf_sb[:])
                    nc.vector.tensor_scalar_mul(h0a_sb[:, t, :], ptt[:], ALPHA)
                    nc.vector.tensor_scalar(
                        out=g_sb[:, t, :], in0=ptt[:],
                        scalar1=dinv_sb[:, t:t + 1], scalar2=None,
                        op0=mybir.AluOpType.mult)
                    bounce_tile(0, t)

            def emit_ag(k):
                nc.gpsimd.collective_compute(
                    "AllGather", mybir.AluOpType.bypass,
                    replica_groups=[list(range(M))],
                    ins=[bounce[k].opt()], outs=[table[k].opt()])

            emit_ag(0)

            # per-pass column -> tile maps
            colstart = []
            col2tile = []
            for q in range(NQ):
                cs = [0]
                c2t = []
                for t, st in enumerate(S[q]):
                    cs.append(cs[-1] + st)
                    c2t += [t] * st
                colstart.append(cs)
                col2tile.append(c2t)
            # gather idx base (in 16-wrap columns) per pass
            gbase = [0]
            for q in range(NQ):
                gbase.append(gbase[-1] + 128 * TCq[q] // 16)
            # scatter chunking (tiles)
            sch = []
            t0 = 0
            while t0 < NT:
                nt_ = min(24, NT - t0)
                if NT - (t0 + nt_) == 2:
                    nt_ += 2
                sch.append((t0, nt_))
                t0 += nt_

            # ================= propagation =================
            for k in range(KSTEPS):
                last = k == KSTEPS - 1
                # zero acc[k] (early, off critical path)
                for t in range(NT):
                    nc.sync.dma_start(
                        out=acc[k][:, :].rearrange("(t p) c -> p t c", p=128)[:, t, :],
                        in_=zerof_sb[:])
                def emit_scatters(qq):
                    st_ = stash[qq % 2]
                    for (t0_, nt_) in sch:
                        nn = 128 * nt_
                        nc.gpsimd.dma_scatter_add(
                            out_ap=acc[k][:, :],
                            in_ap=st_[:, t0_:t0_ + nt_, :],
                            idxs_ap=sidx_sb[:, (qq * NSP + t0_ * 128) // 16:(qq * NSP + (t0_ + nt_) * 128) // 16],
                            num_idxs=nn,
                            num_idxs_reg=nn,
                            elem_size=C,
                        )

                # scatters for pass q are emitted AFTER pass q+1's gathers so
                # they never stall the Pool queue waiting on pass-q compute
                for q in range(NQ):
                    st_buf = stash[q % 2]
                    pag = None
                    for g0 in range(0, TCq[q], GCH):
                        gw = min(GCH, TCq[q] - g0)
                        gt = gatp.tile([128, GCH, 2 * C], bf16, tag="gt")
                        nc.gpsimd.dma_gather(
                            out_ap=gt[:, :gw, :],
                            in_ap=table[k][q * QROWS:(q + 1) * QROWS, :],
                            idxs_ap=gidx_sb[:, gbase[q] + g0 * 8:gbase[q] + (g0 + gw) * 8],
                            num_idxs=128 * gw,
                            num_idxs_reg=128 * gw,
                            elem_size=2 * C,
                        )
                        if g0 == 0 and q > 0:
                            emit_scatters(q - 1)
                        for j in range(gw):
                            col = g0 + j
                            t = col2tile[q][col]
                            s = col - colstart[q][t]
                            if s == 0:
                                pag = psa.tile([128, C], fp32, tag="pag")
                            nc.tensor.matmul(
                                pag[:], lhsT=id_sb[:], rhs=gt[:, j, :C],
                                start=(s == 0), stop=(s == S[q][t] - 1))
                            if s == S[q][t] - 1:
                                nc.vector.tensor_copy(out=st_buf[:, t, :], in_=pag[:])
                emit_scatters(NQ - 1)
                # epilogue: h_new = dinv09*(acc + g_own) + h0a
                for t in range(NT):
                    at = epip.tile([128, C], fp32, tag="at")
                    nc.sync.dma_start(
                        out=at[:], in_=acc[k][t * 128:(t + 1) * 128, :])
                    t1 = epip.tile([128, C], fp32, tag="t1")
                    nc.vector.tensor_add(t1[:], at[:], g_sb[:, t, :])
                    hn = epip.tile([128, C], fp32, tag="hn")
                    nc.vector.tensor_scalar(
                        out=hn[:], in0=t1[:], scalar1=dinv09_sb[:, t:t + 1],
                        scalar2=None, op0=mybir.AluOpType.mult)
                    nc.vector.tensor_add(hn[:], hn[:], h0a_sb[:, t, :])
                    if last:
                        nc.sync.dma_start(out=out_e[t * 128:(t + 1) * 128, :], in_=hn[:])
                    else:
                        nc.vector.tensor_scalar(
                            out=g_sb[:, t, :], in0=hn[:],
                            scalar1=dinv_sb[:, t:t + 1], scalar2=None,
                            op0=mybir.AluOpType.mult)
                        bounce_tile(k + 1, t)
                if not last:
                    emit_ag(1)
    # after Tile sem assignment, pin each SWDGE instruction's queue to the
    # DMASW lane it was assigned (sem lanes are locked to one queue each)
    from concourse.tile_sem_assignment import PROC_NAME_TO_IDX
    lane_by_proc = {PROC_NAME_TO_IDX[f"DMASW{i}"]: i for i in range(8)}
    for inst in nc.all_instructions():
        if isinstance(inst, (mybir.InstDMAGatherAnt, mybir.InstDMAScatterAddAnt)):
            proc = getattr(inst, "bass_scheduled_proc", None)
            if proc in lane_by_proc:
                inst.queue_num = lane_by_proc[proc] % 4
    nc.compile()
    return nc


def kernel(x, W1, b1, p, W2, b2, edge_index):
    from concourse.bass_utils import run_bass_kernel_spmd

    NSH, NSP, NT, TBL, QROWS = _derived()
    in_maps, sched = _host_prep(
        np.asarray(x, dtype=np.float32), np.asarray(W1, dtype=np.float32),
        np.asarray(b1, dtype=np.float32), np.asarray(p, dtype=np.float32),
        np.asarray(W2, dtype=np.float32), np.asarray(b2, dtype=np.float32),
        np.asarray(edge_index))
    nc = _build(sched)
    res = run_bass_kernel_spmd(nc, in_maps, list(range(M)))
    out = np.empty((N, C), dtype=np.float32)
    for c in range(M):
        out[c * NSH:(c + 1) * NSH, :] = res.results[c]["out"][:NSH, :]
    return out


# revision 14
# speedup vs baseline: 1.0038x; 1.0038x over previous
"""APPNP (MLP + 2-step GCN propagation) on 8 Trainium2 NeuronCores.

Strategy (data-parallel over nodes):
 - nodes sharded 12500/core (padded to 12544), natural order.
 - MLP runs transposed (features on partitions) in bf16, PSUM fp32.
 - propagation: g = dinv * h is AllGathered (bf16, rows padded to 256 B)
   into a replicated DRAM table [8*12544, 128]; each core gathers g[src]
   for its edges with batched SWDGE dma_gather (int16 indices).
 - int16 index range forces 4 source-quarter passes (25088 rows each).
   Each pass uses its OWN destination permutation (dsts sorted by that
   quarter's in-count) so the padded-CSR slot schedule stays tight; the
   pass result is merged into a DRAM accumulator with dma_scatter_add
   (indices are core-local, int16-safe), which also un-permutes.
 - h_{k+1} = 0.9*dinv*(acc + g_own) + 0.1*h0 per 128-node tile on DVE.
"""

import numpy as np
import ml_dtypes

N, E, F, H, C = 100000, 1600000, 512, 256, 64
KSTEPS, ALPHA = 2, 0.1
M = 8                       # cores
NQ = 4                      # source quarters (int16 gather range)
GCH = 8                     # gather chunk: 8 cols * 128 = 1024 idxs (HW cap)
BF16 = ml_dtypes.bfloat16


def _derived():
    NSH = N // M                          # real nodes per core
    NSP = ((NSH + 127) // 128) * 128      # padded
    NT = NSP // 128                       # tiles per core
    TBL = M * NSP                         # table rows
    QROWS = TBL // NQ                     # rows per source quarter
    return NSH, NSP, NT, TBL, QROWS


def _wrap16(idx):
    """idx[i] -> [128, ceil(n/16)] int16, entry i at (i%16, i//16),
    replicated across the 8 16-partition stripes."""
    n = idx.size
    cols = (n + 15) // 16
    a = np.zeros((16, cols), dtype=np.int16)
    a[np.arange(n) % 16, np.arange(n) // 16] = idx.astype(np.int16)
    return np.tile(a, (8, 1))


def _host_prep(x, W1, b1, p, W2, b2, edge_index):
    NSH, NSP, NT, TBL, QROWS = _derived()
    src = edge_index[0].astype(np.int64)
    dst = edge_index[1].astype(np.int64)

    deg = (np.bincount(dst, minlength=N) + 1).astype(np.float32)

    # table row of a global src node (natural per-core order, padded)
    src_row = (src // NSH) * NSP + (src % NSH)
    src_q = src_row // QROWS
    src_local = (src_row - src_q * QROWS).astype(np.int64)
    dst_core = dst // NSH
    dst_local = (dst % NSH).astype(np.int64)

    # per-(core, quarter) in-counts  [M, NQ, NSP]
    cnt = np.zeros((M, NQ, NSP), dtype=np.int64)
    np.add.at(cnt, (dst_core, src_q, dst_local), 1)

    # per-(core, quarter) dst permutation: sort by count desc (stable)
    perm = np.empty((M, NQ, NSP), dtype=np.int64)
    csort = np.empty((M, NQ, NSP), dtype=np.int64)
    for c in range(M):
        for q in range(NQ):
            pq = np.argsort(-cnt[c, q], kind="stable")
            perm[c, q] = pq
            csort[c, q] = cnt[c, q][pq]

    # shared slot schedule per pass: S[q][t] = max over cores of the
    # (t*128)-th largest count (sorted desc => max within tile at head)
    S = np.maximum(csort[:, :, ::128].max(axis=0), 1)  # [NQ, NT]
    colstart = np.zeros((NQ, NT + 1), dtype=np.int64)
    colstart[:, 1:] = np.cumsum(S, axis=1)
    TCq = colstart[:, -1]                               # [NQ]

    zero_local = NSP + NSH        # a guaranteed-zero table row per quarter

    # bucket edges by (core, quarter, dst) and emit slot-padded gather idxs
    in_maps = []
    for c in range(M):
        sel = dst_core == c
        sq, sl, dl = src_q[sel], src_local[sel], dst_local[sel]
        gidx_parts = []
        sidx_parts = []
        for q in range(NQ):
            qs = sq == q
            dq, lq = dl[qs], sl[qs]
            pos = np.empty(NSP, dtype=np.int64)
            pos[perm[c, q]] = np.arange(NSP)           # final node -> pass pos
            dpos = pos[dq]                              # pass position of dst
            order = np.argsort(dpos, kind="stable")
            dpos_s, lq_s = dpos[order], lq[order]
            cct = np.bincount(dpos_s, minlength=NSP)
            start = np.concatenate([[0], np.cumsum(cct)])[:-1]
            slot = np.arange(dpos_s.size) - start[dpos_s]
            tiles = dpos_s // 128
            rows = dpos_s % 128
            cols = colstart[q][tiles] + slot
            arr = np.full((int(TCq[q]), 128), zero_local, dtype=np.int64)
            arr[cols, rows] = lq_s                     # idx i = col*128+row
            gidx_parts.append(arr.reshape(-1))
            sidx_parts.append(perm[c, q])              # payload i -> final row
        gidx = _wrap16(np.concatenate(gidx_parts))
        sidx = _wrap16(np.concatenate(sidx_parts))

        lo = c * NSH
        xt = np.zeros((F, NSP), dtype=BF16)
        xt[:, :NSH] = np.ascontiguousarray(x[lo:lo + NSH, :].T).astype(BF16)
        deg_pad = np.ones(NSP, dtype=np.float32)
        deg_pad[:NSH] = deg[lo:lo + NSH]

        in_maps.append({
            "xt": xt,
            "gidx": gidx,
            "sidx": sidx,
            "deg": np.ascontiguousarray(deg_pad.reshape(NT, 128).T),
            "w1": W1.astype(BF16),
            "w2": W2.astype(BF16),
            "pvec": p.astype(np.float32).reshape(2, H // 2).T.copy(),
            "b1": b1.astype(np.float32).reshape(2, H // 2).T.copy(),
            "b2": b2.astype(np.float32).reshape(C, 1).copy(),
            "ident": np.eye(128, dtype=BF16),
            "identf": np.eye(64, dtype=np.float32),
        })

    sched = {
        "S": [[int(v) for v in S[q]] for q in range(NQ)],
        "TCq": [int(v) for v in TCq],
    }
    return in_maps, sched


def _build(sched):
    import concourse.bacc as bacc
    import concourse.mybir as mybir
    import concourse.tile as tile
    from concourse import library_config

    NSH, NSP, NT, TBL, QROWS = _derived()
    S = sched["S"]
    TCq = sched["TCq"]
    GTOT = 128 * sum(TCq)
    fp32 = mybir.dt.float32
    bf16 = mybir.dt.bfloat16
    i16 = mybir.dt.int16

    nc = bacc.Bacc(None, target_bir_lowering=False, num_swdge_queues=4)

    xt_e = nc.declare_dram_parameter("xt", [F, NSP], bf16, isOutput=False)
    gidx_e = nc.declare_dram_parameter("gidx", [128, GTOT // 16], i16, isOutput=False)
    sidx_e = nc.declare_dram_parameter("sidx", [128, NQ * NSP // 16], i16, isOutput=False)
    deg_e = nc.declare_dram_parameter("deg", [128, NT], fp32, isOutput=False)
    w1_e = nc.declare_dram_parameter("w1", [F, H], bf16, isOutput=False)
    w2_e = nc.declare_dram_parameter("w2", [H, C], bf16, isOutput=False)
    p_e = nc.declare_dram_parameter("pvec", [H // 2, 2], fp32, isOutput=False)
    b1_e = nc.declare_dram_parameter("b1", [H // 2, 2], fp32, isOutput=False)
    b2_e = nc.declare_dram_parameter("b2", [C, 1], fp32, isOutput=False)
    id_e = nc.declare_dram_parameter("ident", [128, 128], bf16, isOutput=False)
    idf_e = nc.declare_dram_parameter("identf", [64, 64], fp32, isOutput=False)
    out_e = nc.declare_dram_parameter("out", [NSP, C], fp32, isOutput=True)

    # MLP row chunks
    rcs = []
    off = 0
    while off < NSP:
        w = min(512, NSP - off)
        rcs.append((off, w))
        off += w

    with tile.TileContext(nc) as tc:
        with (
            tc.tile_pool(name="const", bufs=1) as constp,
            tc.tile_pool(name="big", bufs=1) as bigp,
            tc.tile_pool(name="xts", bufs=3) as xtp,
            tc.tile_pool(name="acts", bufs=3) as actp,
            tc.tile_pool(name="gat", bufs=8) as gatp,
            tc.tile_pool(name="epi", bufs=8) as epip,
            tc.tile_pool(name="ps1", bufs=2, space="PSUM") as ps1,
            tc.tile_pool(name="ps2", bufs=2, space="PSUM") as ps2,
            tc.tile_pool(name="pst", bufs=2, space="PSUM") as pst,
            tc.tile_pool(name="psa", bufs=2, space="PSUM") as psa,
            tc.tile_pool(name="dram", bufs=1, space="DRAM") as dramp,
        ):
            nc.gpsimd.load_library(library_config.mlp)

            # ---- constants ----
            w1_sb = constp.tile([128, F // 128, H], bf16)
            nc.sync.dma_start(out=w1_sb[:], in_=w1_e.ap().rearrange("(c p) h -> p c h", p=128))
            w2_sb = constp.tile([128, H // 128, C], bf16)
            nc.sync.dma_start(out=w2_sb[:], in_=w2_e.ap().rearrange("(c p) h -> p c h", p=128))
            id_sb = constp.tile([128, 128], bf16)
            nc.sync.dma_start(out=id_sb[:], in_=id_e[:, :])
            idf_sb = constp.tile([64, 64], fp32)
            nc.sync.dma_start(out=idf_sb[:], in_=idf_e[:, :])
            b2_sb = constp.tile([64, 1], fp32)
            nc.sync.dma_start(out=b2_sb[:], in_=b2_e[:, :])

            gidx_sb = constp.tile([128, GTOT // 16], i16)
            nc.sync.dma_start(out=gidx_sb[:], in_=gidx_e[:, :])
            sidx_sb = constp.tile([128, NQ * NSP // 16], i16)
            nc.sync.dma_start(out=sidx_sb[:], in_=sidx_e[:, :])

            p_sb = constp.tile([H // 2, 2], fp32)
            nc.sync.dma_start(out=p_sb[:], in_=p_e[:, :])
            b1_sb = constp.tile([H // 2, 2], fp32)
            nc.sync.dma_start(out=b1_sb[:], in_=b1_e[:, :])
            pc_sb = constp.tile([H // 2, 2], fp32)
            nc.vector.tensor_scalar(
                out=pc_sb[:], in0=p_sb[:], scalar1=0.0, scalar2=1.0,
                op0=mybir.AluOpType.max, op1=mybir.AluOpType.min)
            pb1_sb = constp.tile([H // 2, 2], fp32)
            nc.vector.tensor_mul(out=pb1_sb[:], in0=pc_sb[:], in1=b1_sb[:])

            deg_sb = constp.tile([128, NT], fp32)
            nc.sync.dma_start(out=deg_sb[:], in_=deg_e[:, :])
            sq_sb = constp.tile([128, NT], fp32)
            nc.scalar.sqrt(out=sq_sb[:], in_=deg_sb[:])
            dinv_sb = constp.tile([128, NT], fp32)
            nc.vector.reciprocal(out=dinv_sb[:], in_=sq_sb[:])
            dinv09_sb = constp.tile([128, NT], fp32)
            nc.vector.tensor_scalar_mul(dinv09_sb[:], dinv_sb[:], 1.0 - ALPHA)

            # ---- persistent big buffers ----
            h0a_sb = bigp.tile([128, NT, C], fp32)      # 0.1 * h0
            g_sb = bigp.tile([128, NT, C], bf16)        # current g (own shard)
            stash = [bigp.tile([128, NT, C], fp32, name=f"stash{i}") for i in range(2)]

            # ---- DRAM bounce + tables + accumulators ----
            bounce = [dramp.tile([NSP, 2 * C], bf16, tag=f"bounce{k}", name=f"bounce{k}") for k in range(2)]
            table = [
                dramp.tile([TBL, 2 * C], bf16, tag=f"table{k}", name=f"table{k}", addr_space="Shared")
                for k in range(2)
            ]
            acc = [dramp.tile([NSP, C], fp32, tag=f"acc{k}", name=f"acc{k}") for k in range(2)]

            zero_sb = constp.tile([128, C], bf16, name="zero_sb")
            nc.vector.memset(zero_sb[:], 0.0)
            zerof_sb = constp.tile([128, C], fp32, name="zerof_sb")
            nc.vector.memset(zerof_sb[:], 0.0)

            # one-time zeroing: bounce pad halves (cols C:2C), pad rows
            for k in range(2):
                for t in range(NT):
                    nc.sync.dma_start(
                        out=bounce[k][:, :].rearrange("(t p) c -> p t c", p=128)[:, t, C:],
                        in_=zero_sb[:])
                nc.sync.dma_start(out=bounce[k][NSH:NSP, :C], in_=zero_sb[:NSP - NSH, :])

            def bounce_tile(k, t):
                rows = min(128, NSH - t * 128)
                if rows <= 0:
                    return
                nc.sync.dma_start(
                    out=bounce[k][t * 128:t * 128 + rows, :C],
                    in_=g_sb[:rows, t, :])

            # ================= MLP =================
            for (off, w) in rcs:
                xt_sb = xtp.tile([128, F // 128, 512], bf16, tag="xt")
                nc.sync.dma_start(
                    out=xt_sb[:, :, :w],
                    in_=xt_e[:, off:off + w].rearrange("(c p) n -> p c n", p=128))
                a_sb = actp.tile([128, H // 128, 512], bf16, tag="a")
                for ht in range(H // 128):
                    pt1 = ps1.tile([128, 512], fp32, tag="pt1")
                    for fc in range(F // 128):
                        nc.tensor.matmul(
                            pt1[:, :w],
                            lhsT=w1_sb[:, fc, ht * 128:(ht + 1) * 128],
                            rhs=xt_sb[:, fc, :w],
                            start=(fc == 0), stop=(fc == F // 128 - 1))
                    nc.scalar.activation(
                        out=a_sb[:, ht, :w], in_=pt1[:, :w],
                        func=mybir.ActivationFunctionType.Relu,
                        bias=pb1_sb[:, ht:ht + 1], scale=pc_sb[:, ht:ht + 1])
                pt2 = ps2.tile([C, 512], fp32, tag="pt2")
                for ht in range(H // 128):
                    nc.tensor.matmul(
                        pt2[:, :w], lhsT=w2_sb[:, ht, :], rhs=a_sb[:, ht, :w],
                        start=(ht == 0), stop=(ht == H // 128 - 1))
                h0t_sb = actp.tile([C, 512], fp32, tag="h0t")
                nc.vector.tensor_scalar_add(h0t_sb[:, :w], pt2[:, :w], b2_sb[:, :1])
                for j in range(w // 128):
                    t = off // 128 + j
                    ptt = pst.tile([128, C], fp32, tag="ptt")
                    nc.tensor.transpose(
                        out=ptt[:], in_=h0t_sb[:, j * 128:(j + 1) * 128],
                        identity=idf_sb[:])
                    nc.vector.tensor_scalar_mul(h0a_sb[:, t, :], ptt[:], ALPHA)
                    nc.vector.tensor_scalar(
                        out=g_sb[:, t, :], in0=ptt[:],
                        scalar1=dinv_sb[:, t:t + 1], scalar2=None,
                        op0=mybir.AluOpType.mult)
                    bounce_tile(0, t)

            def emit_ag(k):
                nc.gpsimd.collective_compute(
                    "AllGather", mybir.AluOpType.bypass,
                    replica_groups=[list(range(M))],
                    ins=[bounce[k].opt()], outs=[table[k].opt()])

            emit_ag(0)

            # per-pass column -> tile maps
            colstart = []
            col2tile = []
            for q in range(NQ):
                cs = [0]
                c2t = []
                for t, st in enumerate(S[q]):
                    cs.append(cs[-1] + st)
                    c2t += [t] * st
                colstart.append(cs)
                col2tile.append(c2t)
            # gather idx base (in 16-wrap columns) per pass
            gbase = [0]
            for q in range(NQ):
                gbase.append(gbase[-1] + 128 * TCq[q] // 16)
            # scatter chunking (tiles)
            sch = []
            t0 = 0
            while t0 < NT:
                nt_ = min(24, NT - t0)
                if NT - (t0 + nt_) == 2:
                    nt_ += 2
                sch.append((t0, nt_))
                t0 += nt_

            # ================= propagation =================
            for k in range(KSTEPS):
                last = k == KSTEPS - 1
                # zero acc[k] (early, off critical path)
                for t in range(NT):
                    nc.sync.dma_start(
                        out=acc[k][:, :].rearrange("(t p) c -> p t c", p=128)[:, t, :],
                        in_=zerof_sb[:])
                for q in range(NQ):
                    st_buf = stash[q % 2]
                    pag = None
                    for g0 in range(0, TCq[q], GCH):
                        gw = min(GCH, TCq[q] - g0)
                        gt = gatp.tile([128, GCH, 2 * C], bf16, tag="gt")
                        nc.gpsimd.dma_gather(
                            out_ap=gt[:, :gw, :],
                            in_ap=table[k][q * QROWS:(q + 1) * QROWS, :],
                            idxs_ap=gidx_sb[:, gbase[q] + g0 * 8:gbase[q] + (g0 + gw) * 8],
                            num_idxs=128 * gw,
                            num_idxs_reg=128 * gw,
                            elem_size=2 * C,
                        )
                        for j in range(gw):
                            col = g0 + j
                            t = col2tile[q][col]
                            s = col - colstart[q][t]
                            if s == 0:
                                pag = psa.tile([128, C], fp32, tag="pag")
                            nc.tensor.matmul(
                                pag[:], lhsT=id_sb[:], rhs=gt[:, j, :C],
                                start=(s == 0), stop=(s == S[q][t] - 1))
                            if s == S[q][t] - 1:
                                nc.vector.tensor_copy(out=st_buf[:, t, :], in_=pag[:])
                    for (t0_, nt_) in sch:
                        nn = 128 * nt_
                        nc.gpsimd.dma_scatter_add(
                            out_ap=acc[k][:, :],
                            in_ap=st_buf[:, t0_:t0_ + nt_, :],
                            idxs_ap=sidx_sb[:, (q * NSP + t0_ * 128) // 16:(q * NSP + (t0_ + nt_) * 128) // 16],
                            num_idxs=nn,
                            num_idxs_reg=nn,
                            elem_size=C,
                        )
                # epilogue: h_new = dinv09*(acc + g_own) + h0a
                for t in range(NT):
                    at = epip.tile([128, C], fp32, tag="at")
                    nc.sync.dma_start(
                        out=at[:], in_=acc[k][t * 128:(t + 1) * 128, :])
                    t1 = epip.tile([128, C], fp32, tag="t1")
                    nc.vector.tensor_add(t1[:], at[:], g_sb[:, t, :])
                    hn = epip.tile([128, C], fp32, tag="hn")
                    nc.vector.tensor_scalar(
                        out=hn[:], in0=t1[:], scalar1=dinv09_sb[:, t:t + 1],
                        scalar2=None, op0=mybir.AluOpType.mult)
                    nc.vector.tensor_add(hn[:], hn[:], h0a_sb[:, t, :])
                    if last:
                        nc.sync.dma_start(out=out_e[t * 128:(t + 1) * 128, :], in_=hn[:])
                    else:
                        nc.vector.tensor_scalar(
                            out=g_sb[:, t, :], in0=hn[:],
                            scalar1=dinv_sb[:, t:t + 1], scalar2=None,
                            op0=mybir.AluOpType.mult)
                        bounce_tile(k + 1, t)
                if not last:
                    emit_ag(1)
    # after Tile sem assignment, pin each SWDGE instruction's queue to the
    # DMASW lane it was assigned (sem lanes are locked to one queue each)
    from concourse.tile_sem_assignment import PROC_NAME_TO_IDX
    lane_by_proc = {PROC_NAME_TO_IDX[f"DMASW{i}"]: i for i in range(8)}
    for inst in nc.all_instructions():
        if isinstance(inst, (mybir.InstDMAGatherAnt, mybir.InstDMAScatterAddAnt)):
            proc = getattr(inst, "bass_scheduled_proc", None)
            if proc in lane_by_proc:
                inst.queue_num = lane_by_proc[proc] % 4
    nc.compile()
    return nc


def kernel(x, W1, b1, p, W2, b2, edge_index):
    from concourse.bass_utils import run_bass_kernel_spmd

    NSH, NSP, NT, TBL, QROWS = _derived()
    in_maps, sched = _host_prep(
        np.asarray(x, dtype=np.float32), np.asarray(W1, dtype=np.float32),
        np.asarray(b1, dtype=np.float32), np.asarray(p, dtype=np.float32),
        np.asarray(W2, dtype=np.float32), np.asarray(b2, dtype=np.float32),
        np.asarray(edge_index))
    nc = _build(sched)
    res = run_bass_kernel_spmd(nc, in_maps, list(range(M)))
    out = np.empty((N, C), dtype=np.float32)
    for c in range(M):
        out[c * NSH:(c + 1) * NSH, :] = res.results[c]["out"][:NSH, :]
    return out


# revision 15
# speedup vs baseline: 1.0068x; 1.0031x over previous
"""APPNP (MLP + 2-step GCN propagation) on 8 Trainium2 NeuronCores.

Strategy (data-parallel over nodes):
 - nodes sharded 12500/core (padded to 12544), natural order.
 - MLP runs transposed (features on partitions) in bf16, PSUM fp32.
 - propagation: g = dinv * h is AllGathered (bf16, rows padded to 256 B)
   into a replicated DRAM table [8*12544, 128]; each core gathers g[src]
   for its edges with batched SWDGE dma_gather (int16 indices).
 - int16 index range forces 4 source-quarter passes (25088 rows each).
   Each pass uses its OWN destination permutation (dsts sorted by that
   quarter's in-count) so the padded-CSR slot schedule stays tight; the
   pass result is merged into a DRAM accumulator with dma_scatter_add
   (indices are core-local, int16-safe), which also un-permutes.
 - h_{k+1} = 0.9*dinv*(acc + g_own) + 0.1*h0 per 128-node tile on DVE.
"""

import numpy as np
import ml_dtypes

N, E, F, H, C = 100000, 1600000, 512, 256, 64
KSTEPS, ALPHA = 2, 0.1
M = 8                       # cores
NQ = 4                      # source quarters (int16 gather range)
GCH = 8                     # gather chunk: 8 cols * 128 = 1024 idxs (HW cap)
BF16 = ml_dtypes.bfloat16


def _derived():
    NSH = N // M                          # real nodes per core
    NSP = ((NSH + 127) // 128) * 128      # padded
    NT = NSP // 128                       # tiles per core
    TBL = M * NSP                         # table rows
    QROWS = TBL // NQ                     # rows per source quarter
    return NSH, NSP, NT, TBL, QROWS


def _wrap16(idx):
    """idx[i] -> [128, ceil(n/16)] int16, entry i at (i%16, i//16),
    replicated across the 8 16-partition stripes."""
    n = idx.size
    cols = (n + 15) // 16
    a = np.zeros((16, cols), dtype=np.int16)
    a[np.arange(n) % 16, np.arange(n) // 16] = idx.astype(np.int16)
    return np.tile(a, (8, 1))


def _host_prep(x, W1, b1, p, W2, b2, edge_index):
    NSH, NSP, NT, TBL, QROWS = _derived()
    src = edge_index[0].astype(np.int64)
    dst = edge_index[1].astype(np.int64)

    deg = (np.bincount(dst, minlength=N) + 1).astype(np.float32)

    # table row of a global src node (natural per-core order, padded)
    src_row = (src // NSH) * NSP + (src % NSH)
    src_q = src_row // QROWS
    src_local = (src_row - src_q * QROWS).astype(np.int64)
    dst_core = dst // NSH
    dst_local = (dst % NSH).astype(np.int64)

    # per-(core, quarter) in-counts  [M, NQ, NSP]
    cnt = np.zeros((M, NQ, NSP), dtype=np.int64)
    np.add.at(cnt, (dst_core, src_q, dst_local), 1)

    # per-(core, quarter) dst permutation: sort by count desc (stable)
    perm = np.empty((M, NQ, NSP), dtype=np.int64)
    csort = np.empty((M, NQ, NSP), dtype=np.int64)
    for c in range(M):
        for q in range(NQ):
            pq = np.argsort(-cnt[c, q], kind="stable")
            perm[c, q] = pq
            csort[c, q] = cnt[c, q][pq]

    # shared slot schedule per pass: S[q][t] = max over cores of the
    # (t*128)-th largest count (sorted desc => max within tile at head)
    S = np.maximum(csort[:, :, ::128].max(axis=0), 1)  # [NQ, NT]
    colstart = np.zeros((NQ, NT + 1), dtype=np.int64)
    colstart[:, 1:] = np.cumsum(S, axis=1)
    TCq = colstart[:, -1]                               # [NQ]

    zero_local = NSP + NSH        # a guaranteed-zero table row per quarter

    # bucket edges by (core, quarter, dst) and emit slot-padded gather idxs
    in_maps = []
    for c in range(M):
        sel = dst_core == c
        sq, sl, dl = src_q[sel], src_local[sel], dst_local[sel]
        gidx_parts = []
        sidx_parts = []
        for q in range(NQ):
            qs = sq == q
            dq, lq = dl[qs], sl[qs]
            pos = np.empty(NSP, dtype=np.int64)
            pos[perm[c, q]] = np.arange(NSP)           # final node -> pass pos
            dpos = pos[dq]                              # pass position of dst
            order = np.argsort(dpos, kind="stable")
            dpos_s, lq_s = dpos[order], lq[order]
            cct = np.bincount(dpos_s, minlength=NSP)
            start = np.concatenate([[0], np.cumsum(cct)])[:-1]
            slot = np.arange(dpos_s.size) - start[dpos_s]
            tiles = dpos_s // 128
            rows = dpos_s % 128
            cols = colstart[q][tiles] + slot
            arr = np.full((int(TCq[q]), 128), zero_local, dtype=np.int64)
            arr[cols, rows] = lq_s                     # idx i = col*128+row
            gidx_parts.append(arr.reshape(-1))
            sidx_parts.append(perm[c, q])              # payload i -> final row
        gidx = _wrap16(np.concatenate(gidx_parts))
        sidx = _wrap16(np.concatenate(sidx_parts))

        lo = c * NSH
        xt = np.zeros((F, NSP), dtype=BF16)
        xt[:, :NSH] = np.ascontiguousarray(x[lo:lo + NSH, :].T).astype(BF16)
        deg_pad = np.ones(NSP, dtype=np.float32)
        deg_pad[:NSH] = deg[lo:lo + NSH]

        in_maps.append({
            "xt": xt,
            "gidx": gidx,
            "sidx": sidx,
            "deg": np.ascontiguousarray(deg_pad.reshape(NT, 128).T),
            "w1": W1.astype(BF16),
            "w2": W2.astype(BF16),
            "pvec": p.astype(np.float32).reshape(2, H // 2).T.copy(),
            "b1": b1.astype(np.float32).reshape(2, H // 2).T.copy(),
            "b2": b2.astype(np.float32).reshape(C, 1).copy(),
            "ident": np.eye(128, dtype=BF16),
            "identf": np.eye(64, dtype=np.float32),
        })

    sched = {
        "S": [[int(v) for v in S[q]] for q in range(NQ)],
        "TCq": [int(v) for v in TCq],
    }
    return in_maps, sched


def _build(sched):
    import concourse.bacc as bacc
    import concourse.mybir as mybir
    import concourse.tile as tile
    from concourse import library_config

    NSH, NSP, NT, TBL, QROWS = _derived()
    S = sched["S"]
    TCq = sched["TCq"]
    GTOT = 128 * sum(TCq)
    fp32 = mybir.dt.float32
    bf16 = mybir.dt.bfloat16
    i16 = mybir.dt.int16

    nc = bacc.Bacc(None, target_bir_lowering=False, num_swdge_queues=4)

    xt_e = nc.declare_dram_parameter("xt", [F, NSP], bf16, isOutput=False)
    gidx_e = nc.declare_dram_parameter("gidx", [128, GTOT // 16], i16, isOutput=False)
    sidx_e = nc.declare_dram_parameter("sidx", [128, NQ * NSP // 16], i16, isOutput=False)
    deg_e = nc.declare_dram_parameter("deg", [128, NT], fp32, isOutput=False)
    w1_e = nc.declare_dram_parameter("w1", [F, H], bf16, isOutput=False)
    w2_e = nc.declare_dram_parameter("w2", [H, C], bf16, isOutput=False)
    p_e = nc.declare_dram_parameter("pvec", [H // 2, 2], fp32, isOutput=False)
    b1_e = nc.declare_dram_parameter("b1", [H // 2, 2], fp32, isOutput=False)
    b2_e = nc.declare_dram_parameter("b2", [C, 1], fp32, isOutput=False)
    id_e = nc.declare_dram_parameter("ident", [128, 128], bf16, isOutput=False)
    idf_e = nc.declare_dram_parameter("identf", [64, 64], fp32, isOutput=False)
    out_e = nc.declare_dram_parameter("out", [NSP, C], fp32, isOutput=True)

    # MLP row chunks
    rcs = []
    off = 0
    while off < NSP:
        w = min(512, NSP - off)
        rcs.append((off, w))
        off += w

    with tile.TileContext(nc) as tc:
        with (
            tc.tile_pool(name="const", bufs=1) as constp,
            tc.tile_pool(name="big", bufs=1) as bigp,
            tc.tile_pool(name="xts", bufs=3) as xtp,
            tc.tile_pool(name="acts", bufs=3) as actp,
            tc.tile_pool(name="gat", bufs=14) as gatp,
            tc.tile_pool(name="epi", bufs=8) as epip,
            tc.tile_pool(name="ps1", bufs=2, space="PSUM") as ps1,
            tc.tile_pool(name="ps2", bufs=2, space="PSUM") as ps2,
            tc.tile_pool(name="pst", bufs=2, space="PSUM") as pst,
            tc.tile_pool(name="psa", bufs=2, space="PSUM") as psa,
            tc.tile_pool(name="dram", bufs=1, space="DRAM") as dramp,
        ):
            nc.gpsimd.load_library(library_config.mlp)

            # ---- constants ----
            w1_sb = constp.tile([128, F // 128, H], bf16)
            nc.sync.dma_start(out=w1_sb[:], in_=w1_e.ap().rearrange("(c p) h -> p c h", p=128))
            w2_sb = constp.tile([128, H // 128, C], bf16)
            nc.sync.dma_start(out=w2_sb[:], in_=w2_e.ap().rearrange("(c p) h -> p c h", p=128))
            id_sb = constp.tile([128, 128], bf16)
            nc.sync.dma_start(out=id_sb[:], in_=id_e[:, :])
            idf_sb = constp.tile([64, 64], fp32)
            nc.sync.dma_start(out=idf_sb[:], in_=idf_e[:, :])
            b2_sb = constp.tile([64, 1], fp32)
            nc.sync.dma_start(out=b2_sb[:], in_=b2_e[:, :])

            gidx_sb = constp.tile([128, GTOT // 16], i16)
            nc.sync.dma_start(out=gidx_sb[:], in_=gidx_e[:, :])
            sidx_sb = constp.tile([128, NQ * NSP // 16], i16)
            nc.sync.dma_start(out=sidx_sb[:], in_=sidx_e[:, :])

            p_sb = constp.tile([H // 2, 2], fp32)
            nc.sync.dma_start(out=p_sb[:], in_=p_e[:, :])
            b1_sb = constp.tile([H // 2, 2], fp32)
            nc.sync.dma_start(out=b1_sb[:], in_=b1_e[:, :])
            pc_sb = constp.tile([H // 2, 2], fp32)
            nc.vector.tensor_scalar(
                out=pc_sb[:], in0=p_sb[:], scalar1=0.0, scalar2=1.0,
                op0=mybir.AluOpType.max, op1=mybir.AluOpType.min)
            pb1_sb = constp.tile([H // 2, 2], fp32)
            nc.vector.tensor_mul(out=pb1_sb[:], in0=pc_sb[:], in1=b1_sb[:])

            deg_sb = constp.tile([128, NT], fp32)
            nc.sync.dma_start(out=deg_sb[:], in_=deg_e[:, :])
            sq_sb = constp.tile([128, NT], fp32)
            nc.scalar.sqrt(out=sq_sb[:], in_=deg_sb[:])
            dinv_sb = constp.tile([128, NT], fp32)
            nc.vector.reciprocal(out=dinv_sb[:], in_=sq_sb[:])
            dinv09_sb = constp.tile([128, NT], fp32)
            nc.vector.tensor_scalar_mul(dinv09_sb[:], dinv_sb[:], 1.0 - ALPHA)

            # ---- persistent big buffers ----
            h0a_sb = bigp.tile([128, NT, C], fp32)      # 0.1 * h0
            g_sb = bigp.tile([128, NT, C], bf16)        # current g (own shard)
            stash = [bigp.tile([128, NT, C], fp32, name=f"stash{i}") for i in range(2)]

            # ---- DRAM bounce + tables + accumulators ----
            bounce = [dramp.tile([NSP, 2 * C], bf16, tag=f"bounce{k}", name=f"bounce{k}") for k in range(2)]
            table = [
                dramp.tile([TBL, 2 * C], bf16, tag=f"table{k}", name=f"table{k}", addr_space="Shared")
                for k in range(2)
            ]
            acc = [dramp.tile([NSP, C], fp32, tag=f"acc{k}", name=f"acc{k}") for k in range(2)]

            zero_sb = constp.tile([128, C], bf16, name="zero_sb")
            nc.vector.memset(zero_sb[:], 0.0)
            zerof_sb = constp.tile([128, C], fp32, name="zerof_sb")
            nc.vector.memset(zerof_sb[:], 0.0)

            # one-time zeroing: bounce pad halves (cols C:2C), pad rows
            for k in range(2):
                for t in range(NT):
                    nc.sync.dma_start(
                        out=bounce[k][:, :].rearrange("(t p) c -> p t c", p=128)[:, t, C:],
                        in_=zero_sb[:])
                nc.sync.dma_start(out=bounce[k][NSH:NSP, :C], in_=zero_sb[:NSP - NSH, :])

            def bounce_tile(k, t):
                rows = min(128, NSH - t * 128)
                if rows <= 0:
                    return
                nc.sync.dma_start(
                    out=bounce[k][t * 128:t * 128 + rows, :C],
                    in_=g_sb[:rows, t, :])

            # ================= MLP =================
            for (off, w) in rcs:
                xt_sb = xtp.tile([128, F // 128, 512], bf16, tag="xt")
                nc.sync.dma_start(
                    out=xt_sb[:, :, :w],
                    in_=xt_e[:, off:off + w].rearrange("(c p) n -> p c n", p=128))
                a_sb = actp.tile([128, H // 128, 512], bf16, tag="a")
                for ht in range(H // 128):
                    pt1 = ps1.tile([128, 512], fp32, tag="pt1")
                    for fc in range(F // 128):
                        nc.tensor.matmul(
                            pt1[:, :w],
                            lhsT=w1_sb[:, fc, ht * 128:(ht + 1) * 128],
                            rhs=xt_sb[:, fc, :w],
                            start=(fc == 0), stop=(fc == F // 128 - 1))
                    nc.scalar.activation(
                        out=a_sb[:, ht, :w], in_=pt1[:, :w],
                        func=mybir.ActivationFunctionType.Relu,
                        bias=pb1_sb[:, ht:ht + 1], scale=pc_sb[:, ht:ht + 1])
                pt2 = ps2.tile([C, 512], fp32, tag="pt2")
                for ht in range(H // 128):
                    nc.tensor.matmul(
                        pt2[:, :w], lhsT=w2_sb[:, ht, :], rhs=a_sb[:, ht, :w],
                        start=(ht == 0), stop=(ht == H // 128 - 1))
                h0t_sb = actp.tile([C, 512], fp32, tag="h0t")
                nc.vector.tensor_scalar_add(h0t_sb[:, :w], pt2[:, :w], b2_sb[:, :1])
                for j in range(w // 128):
                    t = off // 128 + j
                    ptt = pst.tile([128, C], fp32, tag="ptt")
                    nc.tensor.transpose(
                        out=ptt[:], in_=h0t_sb[:, j * 128:(j + 1) * 128],
                        identity=idf_sb[:])
                    nc.vector.tensor_scalar_mul(h0a_sb[:, t, :], ptt[:], ALPHA)
                    nc.vector.tensor_scalar(
                        out=g_sb[:, t, :], in0=ptt[:],
                        scalar1=dinv_sb[:, t:t + 1], scalar2=None,
                        op0=mybir.AluOpType.mult)
                    bounce_tile(0, t)

            def emit_ag(k):
                nc.gpsimd.collective_compute(
                    "AllGather", mybir.AluOpType.bypass,
                    replica_groups=[list(range(M))],
                    ins=[bounce[k].opt()], outs=[table[k].opt()])

            emit_ag(0)

            # per-pass column -> tile maps
            colstart = []
            col2tile = []
            for q in range(NQ):
                cs = [0]
                c2t = []
                for t, st in enumerate(S[q]):
                    cs.append(cs[-1] + st)
                    c2t += [t] * st
                colstart.append(cs)
                col2tile.append(c2t)
            # gather idx base (in 16-wrap columns) per pass
            gbase = [0]
            for q in range(NQ):
                gbase.append(gbase[-1] + 128 * TCq[q] // 16)
            # scatter chunking (tiles)
            sch = []
            t0 = 0
            while t0 < NT:
                nt_ = min(24, NT - t0)
                if NT - (t0 + nt_) == 2:
                    nt_ += 2
                sch.append((t0, nt_))
                t0 += nt_

            # ================= propagation =================
            for k in range(KSTEPS):
                last = k == KSTEPS - 1
                # zero acc[k] (early, off critical path)
                for t in range(NT):
                    nc.sync.dma_start(
                        out=acc[k][:, :].rearrange("(t p) c -> p t c", p=128)[:, t, :],
                        in_=zerof_sb[:])
                for q in range(NQ):
                    st_buf = stash[q % 2]
                    pag = None
                    for g0 in range(0, TCq[q], GCH):
                        gw = min(GCH, TCq[q] - g0)
                        gt = gatp.tile([128, GCH, 2 * C], bf16, tag="gt")
                        nc.gpsimd.dma_gather(
                            out_ap=gt[:, :gw, :],
                            in_ap=table[k][q * QROWS:(q + 1) * QROWS, :],
                            idxs_ap=gidx_sb[:, gbase[q] + g0 * 8:gbase[q] + (g0 + gw) * 8],
                            num_idxs=128 * gw,
                            num_idxs_reg=128 * gw,
                            elem_size=2 * C,
                        )
                        for j in range(gw):
                            col = g0 + j
                            t = col2tile[q][col]
                            s = col - colstart[q][t]
                            if s == 0:
                                pag = psa.tile([128, C], fp32, tag="pag")
                            nc.tensor.matmul(
                                pag[:], lhsT=id_sb[:], rhs=gt[:, j, :C],
                                start=(s == 0), stop=(s == S[q][t] - 1))
                            if s == S[q][t] - 1:
                                nc.vector.tensor_copy(out=st_buf[:, t, :], in_=pag[:])
                    for (t0_, nt_) in sch:
                        nn = 128 * nt_
                        nc.gpsimd.dma_scatter_add(
                            out_ap=acc[k][:, :],
                            in_ap=st_buf[:, t0_:t0_ + nt_, :],
                            idxs_ap=sidx_sb[:, (q * NSP + t0_ * 128) // 16:(q * NSP + (t0_ + nt_) * 128) // 16],
                            num_idxs=nn,
                            num_idxs_reg=nn,
                            elem_size=C,
                        )
                # epilogue: h_new = dinv09*(acc + g_own) + h0a
                for t in range(NT):
                    at = epip.tile([128, C], fp32, tag="at")
                    nc.sync.dma_start(
                        out=at[:], in_=acc[k][t * 128:(t + 1) * 128, :])
                    t1 = epip.tile([128, C], fp32, tag="t1")
                    nc.vector.tensor_add(t1[:], at[:], g_sb[:, t, :])
                    hn = epip.tile([128, C], fp32, tag="hn")
                    nc.vector.tensor_scalar(
                        out=hn[:], in0=t1[:], scalar1=dinv09_sb[:, t:t + 1],
                        scalar2=None, op0=mybir.AluOpType.mult)
                    nc.vector.tensor_add(hn[:], hn[:], h0a_sb[:, t, :])
                    if last:
                        nc.sync.dma_start(out=out_e[t * 128:(t + 1) * 128, :], in_=hn[:])
                    else:
                        nc.vector.tensor_scalar(
                            out=g_sb[:, t, :], in0=hn[:],
                            scalar1=dinv_sb[:, t:t + 1], scalar2=None,
                            op0=mybir.AluOpType.mult)
                        bounce_tile(k + 1, t)
                if not last:
                    emit_ag(1)
    # after Tile sem assignment, pin each SWDGE instruction's queue to the
    # DMASW lane it was assigned (sem lanes are locked to one queue each)
    from concourse.tile_sem_assignment import PROC_NAME_TO_IDX
    lane_by_proc = {PROC_NAME_TO_IDX[f"DMASW{i}"]: i for i in range(8)}
    for inst in nc.all_instructions():
        if isinstance(inst, (mybir.InstDMAGatherAnt, mybir.InstDMAScatterAddAnt)):
            proc = getattr(inst, "bass_scheduled_proc", None)
            if proc in lane_by_proc:
                inst.queue_num = lane_by_proc[proc] % 4
    nc.compile()
    return nc


def kernel(x, W1, b1, p, W2, b2, edge_index):
    from concourse.bass_utils import run_bass_kernel_spmd

    NSH, NSP, NT, TBL, QROWS = _derived()
    in_maps, sched = _host_prep(
        np.asarray(x, dtype=np.float32), np.asarray(W1, dtype=np.float32),
        np.asarray(b1, dtype=np.float32), np.asarray(p, dtype=np.float32),
        np.asarray(W2, dtype=np.float32), np.asarray(b2, dtype=np.float32),
        np.asarray(edge_index))
    nc = _build(sched)
    res = run_bass_kernel_spmd(nc, in_maps, list(range(M)))
    out = np.empty((N, C), dtype=np.float32)
    for c in range(M):
        out[c * NSH:(c + 1) * NSH, :] = res.results[c]["out"][:NSH, :]
    return out
